# revision 1
# baseline (speedup 1.0000x reference)
"""Trainium2 Bass kernel for nn_MultiHeadAttention_89489938580154.

Multi-head attention with a 64-token memory KV prefix, RoPE on self q/k,
causal self-attention, fp32 I/O.  B=4, L=2048, D=216, H=4, hd=54, M=64.

Sharding: 8 cores = 4 batches x 2 head-groups (2 heads each).  Each core
computes its batch/head-group attention and a partial o_proj; the host sums
the two partials per batch (tensor-parallel all-reduce done at gather time).

Self-contained: hardcodes all shapes; builds inputs per core on host
(transposes, head-sliced padded weights, RoPE tables), runs one SPMD Bass
program on cores 0-7 via concourse.bass_utils.run_bass_kernel_spmd.
"""

import os
import numpy as np

B, L, D = 4, 2048, 216
H, HD, HHD = 4, 54, 27
MEM = 64
NCORES = 8
SPAN = 512
NSPAN = L // SPAN            # 4
KCH = 128                    # kv chunk
NKCH = L // KCH              # 16
GROUP = 2                    # kv chunks per PSUM scores tile / exp op
ROPE_THETA = 10000.0

_PROGRAM = None


def _build_program(reps=1):
    from concourse import bass, bacc, mybir
    from concourse import tile
    from concourse import library_config

    FP = mybir.dt.float32
    FPR = mybir.dt.float32r
    Exp = mybir.ActivationFunctionType.Exp

    nc = bacc.Bacc(None, target_bir_lowering=False, debug=False)

    # ---- DRAM parameters (per-core data, host-prepared)
    d_xT = nc.dram_tensor("xT", [D, L], FPR, kind="ExternalInput").ap()
    d_wq = nc.dram_tensor("wq", [D, 128], FPR, kind="ExternalInput").ap()
    d_wk = nc.dram_tensor("wk", [D, 128], FPR, kind="ExternalInput").ap()
    d_wqr = nc.dram_tensor("wqr", [D, 128], FPR, kind="ExternalInput").ap()
    d_wkr = nc.dram_tensor("wkr", [D, 128], FPR, kind="ExternalInput").ap()
    d_wv = nc.dram_tensor("wv", [D, 256], FPR, kind="ExternalInput").ap()
    d_wo = nc.dram_tensor("wo", [128, 256], FPR, kind="ExternalInput").ap()
    d_cos = nc.dram_tensor("cosT", [HD, L], FP, kind="ExternalInput").ap()
    d_sin = nc.dram_tensor("sinT", [HD, L], FP, kind="ExternalInput").ap()
    d_mkT = nc.dram_tensor("memkT", [128, 128], FPR, kind="ExternalInput").ap()
    d_mv = nc.dram_tensor("memv", [MEM, 128], FPR, kind="ExternalInput").ap()
    d_tri = nc.dram_tensor("tri01", [128, 128], FPR, kind="ExternalInput").ap()
    d_out = nc.dram_tensor("outp", [L, D], FP, kind="ExternalOutput").ap()

    with tile.TileContext(nc) as tc:
      nc.gpsimd.load_library(library_config.proxy)
      for _rep in range(reps):
        with tc.tile_pool(name="const", bufs=1) as const, \
             tc.tile_pool(name="work", bufs=2) as work, \
             tc.tile_pool(name="probsp", bufs=4) as probsp:

            # ---------- persistent SBUF tiles + input DMA
            xTa = const.tile([128, L], FPR, tag="xTa")
            xTb = const.tile([88, L], FPR, tag="xTb")
            for s in range(NSPAN):
                sl = slice(s * SPAN, (s + 1) * SPAN)
                nc.sync.dma_start(out=xTa[:, sl], in_=d_xT[0:128, sl])
                nc.sync.dma_start(out=xTb[:, sl], in_=d_xT[128:216, sl])

            wts = {}
            for nm, dr in (("wq", d_wq), ("wk", d_wk), ("wqr", d_wqr),
                           ("wkr", d_wkr)):
                a = const.tile([128, 128], FPR, tag=nm + "a", name=nm + "a")
                b = const.tile([88, 128], FPR, tag=nm + "b", name=nm + "b")
                nc.sync.dma_start(out=a[:, :], in_=dr[0:128, :])
                nc.sync.dma_start(out=b[:, :], in_=dr[128:216, :])
                wts[nm] = (a, b)
            wva = const.tile([128, 256], FPR, tag="wva")
            wvb = const.tile([88, 256], FPR, tag="wvb")
            nc.sync.dma_start(out=wva[:, :], in_=d_wv[0:128, :])
            nc.sync.dma_start(out=wvb[:, :], in_=d_wv[128:216, :])
            wo_sb = const.tile([128, 256], FPR, tag="wo_sb")
            nc.sync.dma_start(out=wo_sb[:, :], in_=d_wo[:, :])

            cos_sb = const.tile([128, L], FP, tag="cos_sb")
            sin_sb = const.tile([128, L], FP, tag="sin_sb")
            nc.gpsimd.memset(cos_sb[:, :], 0.0)
            nc.gpsimd.memset(sin_sb[:, :], 0.0)
            for base in (0, 64):
                nc.sync.dma_start(out=cos_sb[base:base + HD, :], in_=d_cos[:, :])
                nc.sync.dma_start(out=sin_sb[base:base + HD, :], in_=d_sin[:, :])

            mkT = const.tile([128, 128], FPR, tag="mkT")
            nc.sync.dma_start(out=mkT[:, :], in_=d_mkT[:, :])
            mv = const.tile([MEM, 128], FPR, tag="mv")
            nc.sync.dma_start(out=mv[:, :], in_=d_mv[:, :])
            tri = const.tile([128, 128], FPR, tag="tri")
            nc.sync.dma_start(out=tri[:, :], in_=d_tri[:, :])

            QT = const.tile([128, L], FPR, tag="QT")
            KT = const.tile([128, L], FPR, tag="KT")
            Vg = const.tile([128, NKCH, 128], FPR, tag="Vg")
            AN = const.tile([128, L], FPR, tag="AN")
            nc.gpsimd.memset(AN[:, :].bitcast(FP), 0.0)

            # ---------- QK projection (+rot) and RoPE, per span
            with tc.tile_pool(name="projp", bufs=4, space="PSUM") as projp, \
                 tc.tile_pool(name="vprojp", bufs=2, space="PSUM") as vprojp:
                for s in range(NSPAN):
                    for (wnm, rnm, dstT) in (("wq", "wqr", QT),
                                             ("wk", "wkr", KT)):
                        wa, wb = wts[wnm]
                        ra, rb = wts[rnm]
                        sl = slice(s * SPAN, (s + 1) * SPAN)
                        pp = projp.tile([128, SPAN], FP, tag="proj", name="pp")
                        pr = projp.tile([128, SPAN], FP, tag="proj", name="pr")
                        nc.tensor.matmul(pp[:, :], wa[:, :], xTa[:, sl],
                                         start=True, stop=False)
                        nc.tensor.matmul(pp[:, :], wb[:, :], xTb[:, sl],
                                         start=False, stop=True)
                        nc.tensor.matmul(pr[:, :], ra[:, :], xTa[:, sl],
                                         start=True, stop=False)
                        nc.tensor.matmul(pr[:, :], rb[:, :], xTb[:, sl],
                                         start=False, stop=True)
                        t1 = work.tile([128, SPAN], FP, tag="ropet1", name="t1")
                        t2 = work.tile([128, SPAN], FP, tag="ropet2", name="t2")
                        nc.vector.tensor_mul(t1[:, :], pr[:, :], sin_sb[:, sl])
                        nc.vector.tensor_mul(t2[:, :], pp[:, :], cos_sb[:, sl])
                        nc.vector.tensor_add(dstT[:, sl], t1[:, :], t2[:, :])

                # ---------- V projection (natural layout + ones columns)
                for k in range(NKCH):
                    sl = slice(k * KCH, (k + 1) * KCH)
                    pv = vprojp.tile([128, 256], FP, tag="vproj", name="pv")
                    nc.tensor.matmul(pv[:, :], xTa[:, sl], wva[:, :],
                                     start=True, stop=False)
                    nc.tensor.matmul(pv[:, :], xTb[:, sl], wvb[:, :],
                                     start=False, stop=True)
                    nc.vector.tensor_copy(Vg[:, k, :], pv[:, 0:128])
                    nc.gpsimd.memset(Vg[:, k, 32::64].bitcast(FP), 1.0)

            # ---------- attention + o_proj
            with tc.tile_pool(name="scp", bufs=2, space="PSUM") as scp, \
                 tc.tile_pool(name="accp", bufs=3, space="PSUM") as accp, \
                 tc.tile_pool(name="oprojp", bufs=1, space="PSUM") as oprojp:
                for s in range(NSPAN):
                    qsl = slice(s * SPAN, (s + 1) * SPAN)
                    nself = 4 * s + 4
                    last_ki = nself - 1
                    accs = []
                    for h in range(2):
                        acc = accp.tile([64, SPAN], FP, tag="acc",
                                        name="acc")
                        accs.append(acc)
                    for ki in [-1] + list(range(nself)):
                        sc = scp.tile([128, 2 * SPAN], FP, tag="sc", name="sc")
                        for h in range(2):
                            hb = 64 * h
                            hq = slice(hb, hb + HD)
                            osl = slice(h * SPAN, (h + 1) * SPAN)
                            if ki < 0:
                                nc.tensor.matmul(sc[:, osl], mkT[hq, :],
                                                 QT[hq, qsl],
                                                 start=True, stop=True)
                            else:
                                ksl = slice(ki * KCH, (ki + 1) * KCH)
                                nc.tensor.matmul(sc[:, osl], KT[hq, ksl],
                                                 QT[hq, qsl],
                                                 start=True, stop=True)
                        pb = probsp.tile([128, 2 * SPAN], FPR, tag="probs",
                                         name="pb")
                        jloc = ki - 4 * s          # diag offset (>=0: diag)
                        if ki < 4 * s or jloc == 0:
                            nc.scalar.activation(pb[:, :], sc[:, :], Exp)
                        else:
                            for h in range(2):
                                c0 = h * SPAN
                                nc.scalar.activation(
                                    pb[:, c0 + 128 * jloc:c0 + SPAN],
                                    sc[:, c0 + 128 * jloc:c0 + SPAN], Exp)
                                nc.gpsimd.memset(
                                    pb[:, c0:c0 + 128 * jloc].bitcast(FP),
                                    0.0)
                        if ki >= 4 * s:
                            for h in range(2):
                                c0 = h * SPAN
                                dsl = slice(c0 + 128 * jloc,
                                            c0 + 128 * jloc + 128)
                                nc.gpsimd.tensor_mul(pb[:, dsl], pb[:, dsl],
                                                     tri[:, :])
                        for h in range(2):
                            osl = slice(h * SPAN, (h + 1) * SPAN)
                            if ki < 0:
                                nc.tensor.matmul(accs[h][0:64, :],
                                                 mv[:, 64 * h:64 * h + 64],
                                                 pb[0:64, osl],
                                                 start=True, stop=False)
                            else:
                                nc.tensor.matmul(accs[h][0:64, :],
                                                 Vg[:, ki, 64 * h:64 * h + 64],
                                                 pb[:, osl],
                                                 start=False,
                                                 stop=(ki == last_ki))
                    for h in range(2):
                        hb = 64 * h
                        acc = accs[h]
                        denrow = work.tile([1, SPAN], FP, tag="denrow",
                                           name="denrow")
                        nc.vector.reciprocal(denrow[:, :], acc[32:33, :])
                        denb = work.tile([64, SPAN], FP, tag="denb",
                                         name="denb")
                        nc.gpsimd.partition_broadcast(denb[:, :], denrow[:, :])
                        nc.vector.tensor_mul(AN[hb:hb + 64, qsl],
                                             acc[0:64, :], denb[:, :])
                    # o_proj for this span's q tiles
                    for t in range(SPAN // 128):
                        qt = s * SPAN + t * 128
                        po = oprojp.tile([128, 256], FP, tag="oproj",
                                         name="po")
                        nc.tensor.matmul(po[:, :], AN[:, qt:qt + 128],
                                         wo_sb[:, :], start=True, stop=True)
                        ost = work.tile([128, D], FP, tag="ost", name="ost")
                        nc.vector.tensor_copy(ost[:, :], po[:, 0:D])
                        nc.sync.dma_start(out=d_out[qt:qt + 128, :],
                                          in_=ost[:, :])
    nc.compile()
    return nc


def _host_inputs(x, mem_k, mem_v, Wqkv, Wo):
    """Build the per-core input maps (host-side sharding + layout prep)."""
    f32 = np.float32
    x = np.asarray(x, f32)
    mem_k = np.asarray(mem_k, f32)
    mem_v = np.asarray(mem_v, f32)
    Wqkv = np.asarray(Wqkv, f32)
    Wo = np.asarray(Wo, f32)

    Wq, Wk, Wv = Wqkv[:, 0:D], Wqkv[:, D:2 * D], Wqkv[:, 2 * D:3 * D]
    scale = f32(HD ** -0.5)

    # RoPE tables [54, 2048]; sign of rotate_half folded into sinT
    inv = 1.0 / (ROPE_THETA ** (np.arange(0, HD, 2, dtype=np.float64) / HD))
    t = np.arange(L, dtype=np.float64)
    fr = np.outer(t, inv)                       # [L, 27]
    emb = np.concatenate([fr, fr], axis=-1)     # [L, 54]
    cosT = np.ascontiguousarray(np.cos(emb).T).astype(f32)
    sinT = np.ascontiguousarray(np.sin(emb).T).astype(f32)
    sinT[:HHD] *= -1.0

    rotperm = np.concatenate([np.arange(HHD, HD), np.arange(0, HHD)])
    tri01 = np.triu(np.ones((128, 128), f32))   # keep kv<=q: p<=c

    in_maps = []
    for c in range(NCORES):
        b, hg = c // 2, c % 2
        c0 = hg * 2 * HD                        # first head-dim col

        def padw(w, sc=None):
            out = np.zeros((D, 128), f32)
            blk = w[:, c0:c0 + 2 * HD]
            if sc is not None:
                blk = blk * sc
            out[:, 0:HD] = blk[:, 0:HD]
            out[:, 64:64 + HD] = blk[:, HD:2 * HD]
            return out

        wq_p = padw(Wq, scale)
        wk_p = padw(Wk)
        wqr_p = np.zeros_like(wq_p)
        wkr_p = np.zeros_like(wk_p)
        for base in (0, 64):
            wqr_p[:, base:base + HD] = wq_p[:, base:base + HD][:, rotperm]
            wkr_p[:, base:base + HD] = wk_p[:, base:base + HD][:, rotperm]

        # per-head 64-col block: [V d0:32 | ones-slot | V d32:54 | zeros]
        wv_p = np.zeros((D, 256), f32)
        for hh in range(2):
            hcol = c0 + hh * HD
            wv_p[:, 64 * hh + 0:64 * hh + 32] = Wv[:, hcol:hcol + 32]
            wv_p[:, 64 * hh + 33:64 * hh + 55] = Wv[:, hcol + 32:hcol + HD]

        # rows match AN layout: [d0:32 | dead | d32:54 | dead] per head
        wo_p = np.zeros((128, 256), f32)
        for hh in range(2):
            hrow = c0 + hh * HD
            wo_p[64 * hh + 0:64 * hh + 32, 0:D] = Wo[hrow:hrow + 32, :]
            wo_p[64 * hh + 33:64 * hh + 55, 0:D] = Wo[hrow + 32:hrow + HD, :]

        mkT_p = np.zeros((128, 128), f32)
        mkT_p[0:HD, 0:MEM] = mem_k[b][:, c0:c0 + HD].T
        mkT_p[64:64 + HD, 0:MEM] = mem_k[b][:, c0 + HD:c0 + 2 * HD].T

        mv_p = np.zeros((MEM, 128), f32)
        for hh in range(2):
            hcol = c0 + hh * HD
            mv_p[:, 64 * hh + 0:64 * hh + 32] = mem_v[b][:, hcol:hcol + 32]
            mv_p[:, 64 * hh + 32] = 1.0
            mv_p[:, 64 * hh + 33:64 * hh + 55] = mem_v[b][:, hcol + 32:hcol + HD]

        in_maps.append({
            "xT": np.ascontiguousarray(x[b].T),
            "wq": wq_p, "wk": wk_p, "wqr": wqr_p, "wkr": wkr_p,
            "wv": wv_p, "wo": wo_p,
            "cosT": cosT, "sinT": sinT,
            "memkT": mkT_p, "memv": mv_p, "tri01": tri01,
        })
    return in_maps


def get_program():
    global _PROGRAM
    if _PROGRAM is None:
        _PROGRAM = _build_program()
    return _PROGRAM


def kernel(x, mem_k, mem_v, attention_mask, Wqkv, Wo):
    from concourse.bass_utils import run_bass_kernel_spmd

    nc = get_program()
    in_maps = _host_inputs(x, mem_k, mem_v, Wqkv, Wo)
    trace = bool(int(os.environ.get("KB_TRACE", "0")))
    res = run_bass_kernel_spmd(nc, in_maps, core_ids=list(range(NCORES)),
                               trace=trace)
    if trace and res.exec_time_ns is not None:
        print(f"HW exec time: {res.exec_time_ns} ns")
    parts = [res.results[c]["outp"] for c in range(NCORES)]
    out = np.stack([parts[2 * b] + parts[2 * b + 1] for b in range(B)])
    return out.astype(np.float32)



# revision 5
# speedup vs baseline: 1.4377x; 1.4377x over previous
"""Trainium2 Bass kernel for nn_MultiHeadAttention_89489938580154.

Multi-head attention with a 64-token memory KV prefix, RoPE on self q/k,
causal self-attention, fp32 I/O.  B=4, L=2048, D=216, H=4, hd=54, M=64.

Sharding: 8 cores = 4 batches x 2 head-groups (2 heads each).  Each core
computes its batch/head-group attention and a partial o_proj; the host sums
the two partials per batch (tensor-parallel all-reduce done at gather time).

v2: software-pipelined emission so the PE never stalls on the ACT engine:
 - scores(ki) run 2 iterations ahead of AV(ki); AV reads probs from SBUF so
   score PSUM tiles recycle independently.
 - QK-projection of span s+1, V-projection, and o_proj of span s-1 are
   emitted as PE "filler" work inside span s's attention loop (ACT exp is
   slower per-ki than PE scores+AV, so fillers absorb the gap).
 - masked-out column ranges of diagonal-chunk scores/exp/AV are trimmed
   (F>=256 kept for full-rate fp32r); the causal triangle of the diagonal
   128x128 block is zeroed post-exp with gpsimd affine_select.
 - diagonal AVs are emitted [j1,j2,j3,j0] so the full-width j0 AV is last
   and carries the accumulation-group stop flag.
 - PSUM: 2 banks shared ring (qkproj pp/pr, vproj pv, oproj po)
   + 4 banks scores (2 x [128,1024]) + 2 banks acc (2 x [64,512]) = 8.
"""

import os
import numpy as np

B, L, D = 4, 2048, 216
H, HD, HHD = 4, 54, 27
MEM = 64
NCORES = 8
SPAN = 512
NSPAN = L // SPAN            # 4
KCH = 128                    # kv chunk
NKCH = L // KCH             # 16
ROPE_THETA = 10000.0

_PROGRAM = None


def _build_program(reps=1):
    from concourse import bass, bacc, mybir
    from concourse import tile
    from concourse import library_config

    FP = mybir.dt.float32
    FPR = mybir.dt.float32r
    Exp = mybir.ActivationFunctionType.Exp
    GE = mybir.AluOpType.is_ge

    nc = bacc.Bacc(None, target_bir_lowering=False, debug=False)

    # ---- DRAM parameters (per-core data, host-prepared)
    d_xT = nc.dram_tensor("xT", [D, L], FPR, kind="ExternalInput").ap()
    d_wq = nc.dram_tensor("wq", [D, 128], FPR, kind="ExternalInput").ap()
    d_wk = nc.dram_tensor("wk", [D, 128], FPR, kind="ExternalInput").ap()
    d_wqr = nc.dram_tensor("wqr", [D, 128], FPR, kind="ExternalInput").ap()
    d_wkr = nc.dram_tensor("wkr", [D, 128], FPR, kind="ExternalInput").ap()
    d_wv = nc.dram_tensor("wv", [D, 256], FPR, kind="ExternalInput").ap()
    d_wo = nc.dram_tensor("wo", [128, 256], FPR, kind="ExternalInput").ap()
    d_cos = nc.dram_tensor("cosT", [128, L], FP, kind="ExternalInput").ap()
    d_sin = nc.dram_tensor("sinT", [128, L], FP, kind="ExternalInput").ap()
    d_mkT = nc.dram_tensor("memkT", [128, 128], FPR, kind="ExternalInput").ap()
    d_mv = nc.dram_tensor("memv", [MEM, 128], FPR, kind="ExternalInput").ap()
    d_out = nc.dram_tensor("outp", [L, D], FP, kind="ExternalOutput").ap()

    with tile.TileContext(nc) as tc:
      nc.gpsimd.load_library(library_config.proxy)
      for _rep in range(reps):
        with tc.tile_pool(name="const", bufs=1) as const, \
             tc.tile_pool(name="work", bufs=2) as work, \
             tc.tile_pool(name="probsp", bufs=6) as probsp, \
             tc.tile_pool(name="mmp", bufs=2, space="PSUM") as mmp, \
             tc.tile_pool(name="scp", bufs=2, space="PSUM") as scp, \
             tc.tile_pool(name="accp", bufs=2, space="PSUM") as accp:

            # ---------- persistent SBUF tiles + input DMA
            # cos/sin arrive host-padded to [128, L]; their DMAs ride the
            # Activation engine's HWDGE queue so they overlap the xT/weight
            # DMAs on the sync queue.
            cos_sb = const.tile([128, L], FP, tag="cos_sb")
            sin_sb = const.tile([128, L], FP, tag="sin_sb")
            for s in range(NSPAN):
                sl = slice(s * SPAN, (s + 1) * SPAN)
                nc.scalar.dma_start(out=cos_sb[:, sl], in_=d_cos[:, sl])
                nc.scalar.dma_start(out=sin_sb[:, sl], in_=d_sin[:, sl])

            wts = {}
            for nm, dr in (("wq", d_wq), ("wqr", d_wqr), ("wk", d_wk),
                           ("wkr", d_wkr)):
                a = const.tile([128, 128], FPR, tag=nm + "a", name=nm + "a")
                b = const.tile([88, 128], FPR, tag=nm + "b", name=nm + "b")
                nc.sync.dma_start(out=a[:, :], in_=dr[0:128, :])
                nc.sync.dma_start(out=b[:, :], in_=dr[128:216, :])
                wts[nm] = (a, b)

            xTa = const.tile([128, L], FPR, tag="xTa")
            xTb = const.tile([88, L], FPR, tag="xTb")
            sl0 = slice(0, SPAN)
            nc.sync.dma_start(out=xTa[:, sl0], in_=d_xT[0:128, sl0])
            nc.sync.dma_start(out=xTb[:, sl0], in_=d_xT[128:216, sl0])

            mkT = const.tile([128, 128], FPR, tag="mkT")
            nc.sync.dma_start(out=mkT[:, :], in_=d_mkT[:, :])
            mv = const.tile([MEM, 128], FPR, tag="mv")
            nc.sync.dma_start(out=mv[:, :], in_=d_mv[:, :])

            wva = const.tile([128, 256], FPR, tag="wva")
            wvb = const.tile([88, 256], FPR, tag="wvb")
            nc.sync.dma_start(out=wva[:, :], in_=d_wv[0:128, :])
            nc.sync.dma_start(out=wvb[:, :], in_=d_wv[128:216, :])

            for s in range(1, NSPAN):
                sl = slice(s * SPAN, (s + 1) * SPAN)
                nc.sync.dma_start(out=xTa[:, sl], in_=d_xT[0:128, sl])
                nc.sync.dma_start(out=xTb[:, sl], in_=d_xT[128:216, sl])

            wo_sb = const.tile([128, 256], FPR, tag="wo_sb")
            nc.sync.dma_start(out=wo_sb[:, :], in_=d_wo[:, :])

            QT = const.tile([128, L], FPR, tag="QT")
            KT = const.tile([128, L], FPR, tag="KT")
            Vg = const.tile([128, NKCH, 128], FPR, tag="Vg")
            AN = const.tile([128, L], FPR, tag="AN")

            # ---------- emit helpers ------------------------------------
            def emit_qkproj(s, wnm, rnm, dstT):
                sl = slice(s * SPAN, (s + 1) * SPAN)
                wa, wb = wts[wnm]
                ra, rb = wts[rnm]
                pp = mmp.tile([128, SPAN], FP, tag="mm", name="pp")
                pr = mmp.tile([128, SPAN], FP, tag="mm", name="pr")
                nc.tensor.matmul(pp[:, :], wa[:, :], xTa[:, sl],
                                 start=True, stop=False)
                nc.tensor.matmul(pp[:, :], wb[:, :], xTb[:, sl],
                                 start=False, stop=True)
                nc.tensor.matmul(pr[:, :], ra[:, :], xTa[:, sl],
                                 start=True, stop=False)
                nc.tensor.matmul(pr[:, :], rb[:, :], xTb[:, sl],
                                 start=False, stop=True)
                t1 = work.tile([128, SPAN], FP, tag="ropet1", name="t1")
                t2 = work.tile([128, SPAN], FP, tag="ropet2", name="t2")
                nc.vector.tensor_mul(t1[:, :], pr[:, :], sin_sb[:, sl])
                nc.vector.tensor_mul(t2[:, :], pp[:, :], cos_sb[:, sl])
                nc.vector.tensor_add(dstT[:, sl], t1[:, :], t2[:, :])

            def emit_vproj(k):
                sl = slice(k * KCH, (k + 1) * KCH)
                pv = mmp.tile([128, SPAN], FP, tag="mm", name="pv")
                nc.tensor.matmul(pv[:, 0:256], xTa[:, sl], wva[:, :],
                                 start=True, stop=False)
                nc.tensor.matmul(pv[:, 0:256], xTb[:, sl], wvb[:, :],
                                 start=False, stop=True)
                nc.vector.tensor_copy(Vg[:, k, :], pv[:, 0:128])
                nc.gpsimd.memset(Vg[:, k, 32::64].bitcast(FP), 1.0)

            def emit_oproj(qt):
                po = mmp.tile([128, SPAN], FP, tag="mm", name="po")
                nc.tensor.matmul(po[:, 0:256], AN[:, qt:qt + 128],
                                 wo_sb[:, :], start=True, stop=True)
                ost = work.tile([128, D], FP, tag="ost", name="ost")
                nc.vector.tensor_copy(ost[:, :], po[:, 0:D])
                nc.sync.dma_start(out=d_out[qt:qt + 128, :], in_=ost[:, :])

            def trim(s, ki):
                """(off, F) of the valid/computed q-column range per head."""
                j = ki - 4 * s
                if ki < 0 or j < 0 or j == 0:
                    return 0, SPAN
                if j == 1:
                    return 128, 384
                return 256, 256          # j == 2 and j == 3

            def emit_scores(s, ki, sc):
                qbase = s * SPAN
                off, F = trim(s, ki)
                for h in range(2):
                    hq = slice(64 * h, 64 * h + HD)
                    c0 = h * SPAN
                    osl = slice(c0 + off, c0 + off + F)
                    qsl = slice(qbase + off, qbase + off + F)
                    if ki < 0:
                        nc.tensor.matmul(sc[:, osl], mkT[hq, :], QT[hq, qsl],
                                         start=True, stop=True)
                    else:
                        ksl = slice(ki * KCH, (ki + 1) * KCH)
                        nc.tensor.matmul(sc[:, osl], KT[hq, ksl], QT[hq, qsl],
                                         start=True, stop=True)

            def emit_exp_mask(s, ki, sc, pb):
                j = ki - 4 * s
                if j < 0 or ki < 0 or j == 0:
                    nc.scalar.activation(pb[:, :], sc[:, :], Exp)
                else:
                    for h in range(2):
                        c0 = h * SPAN
                        e0 = 128 * j
                        nc.scalar.activation(pb[:, c0 + e0:c0 + SPAN],
                                             sc[:, c0 + e0:c0 + SPAN], Exp)
                        if j == 3:
                            nc.gpsimd.memset(
                                pb[:, c0 + 256:c0 + 384].bitcast(FP), 0.0)
                if ki >= 4 * s:
                    # zero the upper (future) triangle of the diagonal block
                    for h in range(2):
                        c0 = h * SPAN
                        blk = slice(c0 + 128 * j, c0 + 128 * j + 128)
                        nc.gpsimd.affine_select(
                            out=pb[:, blk], in_=pb[:, blk],
                            compare_op=GE, fill=0.0, base=0,
                            pattern=[[1, 128]], channel_multiplier=-1)

            def emit_av(s, ki, pb, accs, first, last):
                off, F = trim(s, ki)
                for h in range(2):
                    c0 = h * SPAN
                    if ki < 0:
                        nc.tensor.matmul(accs[h][0:64, 0:SPAN],
                                         mv[:, 64 * h:64 * h + 64],
                                         pb[0:64, c0:c0 + SPAN],
                                         start=True, stop=False,
                                         skip_group_check=True)
                    else:
                        nc.tensor.matmul(accs[h][0:64, off:off + F],
                                         Vg[:, ki, 64 * h:64 * h + 64],
                                         pb[:, c0 + off:c0 + off + F],
                                         start=False, stop=last,
                                         skip_group_check=True)

            def emit_norm(s, accs):
                qsl = slice(s * SPAN, (s + 1) * SPAN)
                for h in range(2):
                    den = work.tile([1, SPAN], FP, tag="den", name="den")
                    nc.vector.reciprocal(den[:, :], accs[h][32:33, :])
                    denb = work.tile([64, SPAN], FP, tag="denb", name="denb")
                    nc.gpsimd.partition_broadcast(denb[:, :], den[:, :])
                    nc.vector.tensor_mul(AN[64 * h:64 * h + 64, qsl],
                                         accs[h][0:64, :], denb[:, :])

            # ---------- span-pipelined main loop ------------------------
            emit_qkproj(0, "wq", "wqr", QT)
            emit_qkproj(0, "wk", "wkr", KT)
            emit_vproj(0)

            for s in range(NSPAN):
                fillers = []
                for k in range(4 * s + (1 if s == 0 else 0), 4 * s + 4):
                    fillers.append(lambda k=k: emit_vproj(k))
                if s < NSPAN - 1:
                    fillers.append(lambda s=s: emit_qkproj(s + 1, "wq", "wqr", QT))
                    fillers.append(lambda s=s: emit_qkproj(s + 1, "wk", "wkr", KT))
                if s >= 1:
                    for t in range(4):
                        qt = (s - 1) * SPAN + t * 128
                        fillers.append(lambda qt=qt: emit_oproj(qt))

                kis = [-1] + list(range(4 * s + 4))
                # AV order: diag j0 moved last (full width, carries stop)
                av_order = kis[:-4] + [4 * s + 1, 4 * s + 2, 4 * s + 3, 4 * s]
                n = len(kis)
                accs = [accp.tile([64, SPAN], FP, tag="acc", name="acc")
                        for _ in range(2)]
                pbs = {}
                for idx, ki in enumerate(kis):
                    sc = scp.tile([128, 2 * SPAN], FP, tag="sc", name="sc")
                    emit_scores(s, ki, sc)
                    pb = probsp.tile([128, 2 * SPAN], FPR, tag="probs",
                                     name="pb")
                    pbs[ki] = pb
                    emit_exp_mask(s, ki, sc, pb)
                    if fillers:
                        fillers.pop(0)()
                    if idx >= 2:
                        aki = av_order[idx - 2]
                        emit_av(s, aki, pbs[aki], accs,
                                first=(aki == -1), last=(aki == 4 * s))
                for i in (n - 2, n - 1):
                    aki = av_order[i]
                    emit_av(s, aki, pbs[aki], accs,
                            first=(aki == -1), last=(aki == 4 * s))
                for f in fillers:
                    f()
                emit_norm(s, accs)

            # last span's o_proj tail
            for t in range(4):
                qt = 3 * SPAN + t * 128
                emit_oproj(qt)

    nc.compile()
    return nc


def _host_inputs(x, mem_k, mem_v, Wqkv, Wo):
    """Build the per-core input maps (host-side sharding + layout prep)."""
    f32 = np.float32
    x = np.asarray(x, f32)
    mem_k = np.asarray(mem_k, f32)
    mem_v = np.asarray(mem_v, f32)
    Wqkv = np.asarray(Wqkv, f32)
    Wo = np.asarray(Wo, f32)

    Wq, Wk, Wv = Wqkv[:, 0:D], Wqkv[:, D:2 * D], Wqkv[:, 2 * D:3 * D]
    scale = f32(HD ** -0.5)

    # RoPE tables, host-padded to [128, 2048]: rows 0:54 and 64:118 hold the
    # per-head tables (identical), pad rows zeroed; sign of rotate_half
    # folded into sinT
    inv = 1.0 / (ROPE_THETA ** (np.arange(0, HD, 2, dtype=np.float64) / HD))
    t = np.arange(L, dtype=np.float64)
    fr = np.outer(t, inv)                       # [L, 27]
    emb = np.concatenate([fr, fr], axis=-1)     # [L, 54]
    cos54 = np.ascontiguousarray(np.cos(emb).T).astype(f32)
    sin54 = np.ascontiguousarray(np.sin(emb).T).astype(f32)
    sin54[:HHD] *= -1.0
    cosT = np.zeros((128, L), f32)
    sinT = np.zeros((128, L), f32)
    for base in (0, 64):
        cosT[base:base + HD] = cos54
        sinT[base:base + HD] = sin54

    rotperm = np.concatenate([np.arange(HHD, HD), np.arange(0, HHD)])

    in_maps = []
    for c in range(NCORES):
        b, hg = c // 2, c % 2
        c0 = hg * 2 * HD                        # first head-dim col

        def padw(w, sc=None):
            out = np.zeros((D, 128), f32)
            blk = w[:, c0:c0 + 2 * HD]
            if sc is not None:
                blk = blk * sc
            out[:, 0:HD] = blk[:, 0:HD]
            out[:, 64:64 + HD] = blk[:, HD:2 * HD]
            return out

        wq_p = padw(Wq, scale)
        wk_p = padw(Wk)
        wqr_p = np.zeros_like(wq_p)
        wkr_p = np.zeros_like(wk_p)
        for base in (0, 64):
            wqr_p[:, base:base + HD] = wq_p[:, base:base + HD][:, rotperm]
            wkr_p[:, base:base + HD] = wk_p[:, base:base + HD][:, rotperm]

        # per-head 64-col block: [V d0:32 | ones-slot | V d32:54 | zeros]
        wv_p = np.zeros((D, 256), f32)
        for hh in range(2):
            hcol = c0 + hh * HD
            wv_p[:, 64 * hh + 0:64 * hh + 32] = Wv[:, hcol:hcol + 32]
            wv_p[:, 64 * hh + 33:64 * hh + 55] = Wv[:, hcol + 32:hcol + HD]

        # rows match AN layout: [d0:32 | dead | d32:54 | dead] per head
        wo_p = np.zeros((128, 256), f32)
        for hh in range(2):
            hrow = c0 + hh * HD
            wo_p[64 * hh + 0:64 * hh + 32, 0:D] = Wo[hrow:hrow + 32, :]
            wo_p[64 * hh + 33:64 * hh + 55, 0:D] = Wo[hrow + 32:hrow + HD, :]

        mkT_p = np.zeros((128, 128), f32)
        mkT_p[0:HD, 0:MEM] = mem_k[b][:, c0:c0 + HD].T
        mkT_p[64:64 + HD, 0:MEM] = mem_k[b][:, c0 + HD:c0 + 2 * HD].T

        mv_p = np.zeros((MEM, 128), f32)
        for hh in range(2):
            hcol = c0 + hh * HD
            mv_p[:, 64 * hh + 0:64 * hh + 32] = mem_v[b][:, hcol:hcol + 32]
            mv_p[:, 64 * hh + 32] = 1.0
            mv_p[:, 64 * hh + 33:64 * hh + 55] = mem_v[b][:, hcol + 32:hcol + HD]

        in_maps.append({
            "xT": np.ascontiguousarray(x[b].T),
            "wq": wq_p, "wk": wk_p, "wqr": wqr_p, "wkr": wkr_p,
            "wv": wv_p, "wo": wo_p,
            "cosT": cosT, "sinT": sinT,
            "memkT": mkT_p, "memv": mv_p,
        })
    return in_maps


def get_program():
    global _PROGRAM
    if _PROGRAM is None:
        _PROGRAM = _build_program()
    return _PROGRAM


def kernel(x, mem_k, mem_v, attention_mask, Wqkv, Wo):
    from concourse.bass_utils import run_bass_kernel_spmd

    nc = get_program()
    in_maps = _host_inputs(x, mem_k, mem_v, Wqkv, Wo)
    trace = bool(int(os.environ.get("KB_TRACE", "0")))
    res = run_bass_kernel_spmd(nc, in_maps, core_ids=list(range(NCORES)),
                               trace=trace)
    if trace and res.exec_time_ns is not None:
        print(f"HW exec time: {res.exec_time_ns} ns")
    parts = [res.results[c]["outp"] for c in range(NCORES)]
    out = np.stack([parts[2 * b] + parts[2 * b + 1] for b in range(B)])
    return out.astype(np.float32)


# revision 6
# speedup vs baseline: 1.8280x; 1.2715x over previous
"""Trainium2 Bass kernel for nn_MultiHeadAttention_89489938580154.

Multi-head attention with a 64-token memory KV prefix, RoPE on self q/k,
causal self-attention, fp32 I/O.  B=4, L=2048, D=216, H=4, hd=54, M=64.

Sharding: 8 cores = 4 batches x 2 head-groups (2 heads each).  Each core
computes its batch/head-group attention and a partial o_proj; the host sums
the two partials per batch (tensor-parallel all-reduce done at gather time).

v3: bf16 data path + software-pipelined emission so the PE never stalls:
 - all matmul operands bf16 (PSUM accumulation stays fp32); rel err ~6e-3
   vs the 2e-2 gate.  Halves input DMA and enables full-rate matmuls at
   F=128 (V-proj, j3 diagonal chunk, o_proj at F=216).
 - scores(ki) run 3 iterations ahead of AV(ki); AV reads probs from SBUF
   so score PSUM tiles recycle independently of the AV lag.
 - QK-projection of span s+1, V-projection, and o_proj of span s-1 are
   emitted as PE "filler" work inside span s's attention loop (ACT exp is
   slower per-ki than PE scores+AV; fillers absorb the gap).
 - masked-out column ranges of diagonal-chunk scores/exp/AV are trimmed;
   sc/probs tiles are [128, 2(head), 512] so one strided ACT op covers
   both heads and one gpsimd affine_select masks both causal triangles.
 - diagonal AVs are emitted [j1,j2,j3,j0] so the full-width j0 AV is last
   and carries the accumulation-group stop flag; for s>=1 the first AV is
   full-width av(0) (carries start) so av(mem) need not wait on the
   previous span's accumulator drain.
 - PSUM: 2-bank shared ring (qkproj pp/pr, vproj pv, oproj po)
   + 4 banks scores (2 x [128,2,512]) + 2 banks acc (2 x [64,512]) = 8.
"""

import os
import numpy as np

B, L, D = 4, 2048, 216
H, HD, HHD = 4, 54, 27
MEM = 64
NCORES = 8
SPAN = 512
NSPAN = L // SPAN            # 4
KCH = 128                    # kv chunk
NKCH = L // KCH             # 16
ROPE_THETA = 10000.0

_PROGRAM = None


def _build_program(reps=1):
    from concourse import bass, bacc, mybir
    from concourse import tile
    from concourse import library_config

    FP = mybir.dt.float32
    BF = mybir.dt.bfloat16
    Exp = mybir.ActivationFunctionType.Exp
    GE = mybir.AluOpType.is_ge

    nc = bacc.Bacc(None, target_bir_lowering=False, debug=False)

    # ---- DRAM parameters (per-core data, host-prepared, bf16)
    d_xT = nc.dram_tensor("xT", [D, L], BF, kind="ExternalInput").ap()
    d_wq = nc.dram_tensor("wq", [D, 128], BF, kind="ExternalInput").ap()
    d_wk = nc.dram_tensor("wk", [D, 128], BF, kind="ExternalInput").ap()
    d_wqr = nc.dram_tensor("wqr", [D, 128], BF, kind="ExternalInput").ap()
    d_wkr = nc.dram_tensor("wkr", [D, 128], BF, kind="ExternalInput").ap()
    d_wv = nc.dram_tensor("wv", [D, 128], BF, kind="ExternalInput").ap()
    d_wo = nc.dram_tensor("wo", [128, 216], BF, kind="ExternalInput").ap()
    d_cos = nc.dram_tensor("cosT", [128, L], BF, kind="ExternalInput").ap()
    d_sin = nc.dram_tensor("sinT", [128, L], BF, kind="ExternalInput").ap()
    d_mkT = nc.dram_tensor("memkT", [128, 128], BF, kind="ExternalInput").ap()
    d_mv = nc.dram_tensor("memv", [MEM, 128], BF, kind="ExternalInput").ap()
    d_out = nc.dram_tensor("outp", [L, D], FP, kind="ExternalOutput").ap()

    with tile.TileContext(nc) as tc:
      nc.gpsimd.load_library(library_config.proxy)
      for _rep in range(reps):
        with tc.tile_pool(name="const", bufs=1) as const, \
             tc.tile_pool(name="work", bufs=2) as work, \
             tc.tile_pool(name="probsp", bufs=6) as probsp, \
             tc.tile_pool(name="mmp", bufs=2, space="PSUM") as mmp, \
             tc.tile_pool(name="scp", bufs=2, space="PSUM") as scp, \
             tc.tile_pool(name="accp", bufs=2, space="PSUM") as accp:

            # ---------- persistent SBUF tiles + input DMA
            # cos/sin arrive host-padded to [128, L]; their DMAs ride the
            # Activation engine's HWDGE queue so they overlap the xT/weight
            # DMAs on the sync queue.
            cos_sb = const.tile([128, L], BF, tag="cos_sb")
            sin_sb = const.tile([128, L], BF, tag="sin_sb")
            for s in range(NSPAN):
                sl = slice(s * SPAN, (s + 1) * SPAN)
                nc.scalar.dma_start(out=cos_sb[:, sl], in_=d_cos[:, sl])
                nc.scalar.dma_start(out=sin_sb[:, sl], in_=d_sin[:, sl])

            wts = {}
            for nm, dr in (("wq", d_wq), ("wqr", d_wqr), ("wk", d_wk),
                           ("wkr", d_wkr)):
                a = const.tile([128, 128], BF, tag=nm + "a", name=nm + "a")
                b = const.tile([88, 128], BF, tag=nm + "b", name=nm + "b")
                nc.sync.dma_start(out=a[:, :], in_=dr[0:128, :])
                nc.sync.dma_start(out=b[:, :], in_=dr[128:216, :])
                wts[nm] = (a, b)

            xTa = const.tile([128, L], BF, tag="xTa")
            xTb = const.tile([88, L], BF, tag="xTb")
            sl0 = slice(0, SPAN)
            nc.sync.dma_start(out=xTa[:, sl0], in_=d_xT[0:128, sl0])
            nc.sync.dma_start(out=xTb[:, sl0], in_=d_xT[128:216, sl0])

            mkT = const.tile([128, 128], BF, tag="mkT")
            nc.sync.dma_start(out=mkT[:, :], in_=d_mkT[:, :])
            mv = const.tile([MEM, 128], BF, tag="mv")
            nc.sync.dma_start(out=mv[:, :], in_=d_mv[:, :])

            wva = const.tile([128, 128], BF, tag="wva")
            wvb = const.tile([88, 128], BF, tag="wvb")
            nc.sync.dma_start(out=wva[:, :], in_=d_wv[0:128, :])
            nc.sync.dma_start(out=wvb[:, :], in_=d_wv[128:216, :])

            for s in range(1, NSPAN):
                sl = slice(s * SPAN, (s + 1) * SPAN)
                nc.sync.dma_start(out=xTa[:, sl], in_=d_xT[0:128, sl])
                nc.sync.dma_start(out=xTb[:, sl], in_=d_xT[128:216, sl])

            wo_sb = const.tile([128, 216], BF, tag="wo_sb")
            nc.sync.dma_start(out=wo_sb[:, :], in_=d_wo[:, :])

            QT = const.tile([128, L], BF, tag="QT")
            KT = const.tile([128, L], BF, tag="KT")
            Vg = const.tile([128, NKCH, 128], BF, tag="Vg")
            AN = const.tile([128, L], BF, tag="AN")

            # ---------- emit helpers ------------------------------------
            def emit_qkproj(s, wnm, rnm, dstT):
                sl = slice(s * SPAN, (s + 1) * SPAN)
                wa, wb = wts[wnm]
                ra, rb = wts[rnm]
                pp = mmp.tile([128, SPAN], FP, tag="mm", name="pp")
                pr = mmp.tile([128, SPAN], FP, tag="mm", name="pr")
                nc.tensor.matmul(pp[:, :], wa[:, :], xTa[:, sl],
                                 start=True, stop=False)
                nc.tensor.matmul(pp[:, :], wb[:, :], xTb[:, sl],
                                 start=False, stop=True)
                nc.tensor.matmul(pr[:, :], ra[:, :], xTa[:, sl],
                                 start=True, stop=False)
                nc.tensor.matmul(pr[:, :], rb[:, :], xTb[:, sl],
                                 start=False, stop=True)
                t1 = work.tile([128, SPAN], FP, tag="ropet1", name="t1")
                t2 = work.tile([128, SPAN], FP, tag="ropet2", name="t2")
                nc.vector.tensor_mul(t1[:, :], pr[:, :], sin_sb[:, sl])
                nc.vector.tensor_mul(t2[:, :], pp[:, :], cos_sb[:, sl])
                nc.vector.tensor_add(dstT[:, sl], t1[:, :], t2[:, :])

            def emit_vproj(k):
                sl = slice(k * KCH, (k + 1) * KCH)
                pv = mmp.tile([128, SPAN], FP, tag="mm", name="pv")
                nc.tensor.matmul(pv[:, 0:128], xTa[:, sl], wva[:, :],
                                 start=True, stop=False)
                nc.tensor.matmul(pv[:, 0:128], xTb[:, sl], wvb[:, :],
                                 start=False, stop=True)
                nc.vector.tensor_copy(Vg[:, k, :], pv[:, 0:128])
                nc.gpsimd.memset(Vg[:, k, 32::64].bitcast(BF), 1.0)

            def emit_oproj(qt):
                po = mmp.tile([128, SPAN], FP, tag="mm", name="po")
                nc.tensor.matmul(po[:, 0:216], AN[:, qt:qt + 128],
                                 wo_sb[:, :], start=True, stop=True)
                ost = work.tile([128, D], FP, tag="ost", name="ost")
                nc.vector.tensor_copy(ost[:, :], po[:, 0:D])
                nc.sync.dma_start(out=d_out[qt:qt + 128, :], in_=ost[:, :])

            def trim(s, ki):
                """(off, F) of the computed q-column range per head."""
                j = ki - 4 * s
                if ki < 0 or j < 0 or j == 0:
                    return 0, SPAN
                return 128 * j, SPAN - 128 * j     # j in (1, 2, 3)

            def emit_scores(s, ki, sc):
                qbase = s * SPAN
                off, F = trim(s, ki)
                for h in range(2):
                    hq = slice(64 * h, 64 * h + HD)
                    qsl = slice(qbase + off, qbase + off + F)
                    if ki < 0:
                        nc.tensor.matmul(sc[:, h, off:off + F], mkT[hq, :],
                                         QT[hq, qsl], start=True, stop=True)
                    else:
                        ksl = slice(ki * KCH, (ki + 1) * KCH)
                        nc.tensor.matmul(sc[:, h, off:off + F], KT[hq, ksl],
                                         QT[hq, qsl], start=True, stop=True)

            def emit_exp_mask(s, ki, sc, pb):
                off, F = trim(s, ki)
                nc.scalar.activation(pb[:, :, off:off + F],
                                     sc[:, :, off:off + F], Exp)
                if ki >= 4 * s:
                    j = ki - 4 * s
                    # zero the upper (future) triangle of the diagonal
                    # 128x128 block, both heads in one op
                    nc.gpsimd.affine_select(
                        out=pb[:, :, 128 * j:128 * j + 128],
                        in_=pb[:, :, 128 * j:128 * j + 128],
                        compare_op=GE, fill=0.0, base=0,
                        pattern=[[0, 2], [1, 128]], channel_multiplier=-1)

            def emit_av(s, ki, pb, accs, first, last):
                off, F = trim(s, ki)
                for h in range(2):
                    if ki < 0:
                        nc.tensor.matmul(accs[h][0:64, 0:SPAN],
                                         mv[:, 64 * h:64 * h + 64],
                                         pb[0:64, h, 0:SPAN],
                                         start=first, stop=False,
                                         skip_group_check=True)
                    else:
                        nc.tensor.matmul(accs[h][0:64, off:off + F],
                                         Vg[:, ki, 64 * h:64 * h + 64],
                                         pb[:, h, off:off + F],
                                         start=first, stop=last,
                                         skip_group_check=True)

            def emit_norm(s, accs):
                qsl = slice(s * SPAN, (s + 1) * SPAN)
                for h in range(2):
                    den = work.tile([1, SPAN], FP, tag="den", name="den")
                    nc.vector.reciprocal(den[:, :], accs[h][32:33, :])
                    denb = work.tile([64, SPAN], FP, tag="denb", name="denb")
                    nc.gpsimd.partition_broadcast(denb[:, :], den[:, :])
                    nc.vector.tensor_mul(AN[64 * h:64 * h + 64, qsl],
                                         accs[h][0:64, :], denb[:, :])

            # ---------- span-pipelined main loop ------------------------
            emit_qkproj(0, "wq", "wqr", QT)
            emit_qkproj(0, "wk", "wkr", KT)
            emit_vproj(0)

            AVLAG = 3
            for s in range(NSPAN):
                fillers = []
                for k in range(4 * s + (1 if s == 0 else 0), 4 * s + 4):
                    fillers.append(lambda k=k: emit_vproj(k))
                if s < NSPAN - 1:
                    fillers.append(lambda s=s: emit_qkproj(s + 1, "wq", "wqr", QT))
                    fillers.append(lambda s=s: emit_qkproj(s + 1, "wk", "wkr", KT))
                if s >= 1:
                    for t in range(4):
                        qt = (s - 1) * SPAN + t * 128
                        fillers.append(lambda qt=qt: emit_oproj(qt))

                kis = [-1] + list(range(4 * s + 4))
                # AV order: full-width av(0) first (carries start) when it
                # exists, diag j0 last (full width, carries stop)
                if s == 0:
                    av_order = [-1, 1, 2, 3, 0]
                else:
                    av_order = ([0, -1] + list(range(1, 4 * s)) +
                                [4 * s + 1, 4 * s + 2, 4 * s + 3, 4 * s])
                n = len(kis)
                accs = [accp.tile([64, SPAN], FP, tag="acc", name="acc")
                        for _ in range(2)]
                pbs = {}
                for idx, ki in enumerate(kis):
                    sc = scp.tile([128, 2, SPAN], FP, tag="sc", name="sc")
                    emit_scores(s, ki, sc)
                    pb = probsp.tile([128, 2, SPAN], BF, tag="probs",
                                     name="pb")
                    pbs[ki] = pb
                    emit_exp_mask(s, ki, sc, pb)
                    if fillers:
                        fillers.pop(0)()
                    if idx >= AVLAG:
                        aki = av_order[idx - AVLAG]
                        emit_av(s, aki, pbs[aki], accs,
                                first=(aki == av_order[0]),
                                last=(aki == 4 * s))
                for i in range(n - AVLAG, n):
                    aki = av_order[i]
                    emit_av(s, aki, pbs[aki], accs,
                            first=(aki == av_order[0]), last=(aki == 4 * s))
                for f in fillers:
                    f()
                emit_norm(s, accs)

            # last span's o_proj tail
            for t in range(4):
                qt = 3 * SPAN + t * 128
                emit_oproj(qt)

    nc.compile()
    return nc


def _host_inputs(x, mem_k, mem_v, Wqkv, Wo):
    """Build the per-core input maps (host-side sharding + layout prep)."""
    import ml_dtypes
    f32 = np.float32
    bf16 = ml_dtypes.bfloat16
    x = np.asarray(x, f32)
    mem_k = np.asarray(mem_k, f32)
    mem_v = np.asarray(mem_v, f32)
    Wqkv = np.asarray(Wqkv, f32)
    Wo = np.asarray(Wo, f32)

    Wq, Wk, Wv = Wqkv[:, 0:D], Wqkv[:, D:2 * D], Wqkv[:, 2 * D:3 * D]
    scale = f32(HD ** -0.5)

    # RoPE tables, host-padded to [128, 2048]: rows 0:54 and 64:118 hold the
    # per-head tables (identical), pad rows zeroed; sign of rotate_half
    # folded into sinT
    inv = 1.0 / (ROPE_THETA ** (np.arange(0, HD, 2, dtype=np.float64) / HD))
    t = np.arange(L, dtype=np.float64)
    fr = np.outer(t, inv)                       # [L, 27]
    emb = np.concatenate([fr, fr], axis=-1)     # [L, 54]
    cos54 = np.ascontiguousarray(np.cos(emb).T).astype(f32)
    sin54 = np.ascontiguousarray(np.sin(emb).T).astype(f32)
    sin54[:HHD] *= -1.0
    cosT = np.zeros((128, L), f32)
    sinT = np.zeros((128, L), f32)
    for base in (0, 64):
        cosT[base:base + HD] = cos54
        sinT[base:base + HD] = sin54
    cosT = cosT.astype(bf16)
    sinT = sinT.astype(bf16)

    rotperm = np.concatenate([np.arange(HHD, HD), np.arange(0, HHD)])

    in_maps = []
    for c in range(NCORES):
        b, hg = c // 2, c % 2
        c0 = hg * 2 * HD                        # first head-dim col

        def padw(w, sc=None):
            out = np.zeros((D, 128), f32)
            blk = w[:, c0:c0 + 2 * HD]
            if sc is not None:
                blk = blk * sc
            out[:, 0:HD] = blk[:, 0:HD]
            out[:, 64:64 + HD] = blk[:, HD:2 * HD]
            return out

        wq_p = padw(Wq, scale)
        wk_p = padw(Wk)
        wqr_p = np.zeros_like(wq_p)
        wkr_p = np.zeros_like(wk_p)
        for base in (0, 64):
            wqr_p[:, base:base + HD] = wq_p[:, base:base + HD][:, rotperm]
            wkr_p[:, base:base + HD] = wk_p[:, base:base + HD][:, rotperm]

        # per-head 64-col block: [V d0:32 | ones-slot | V d32:54 | zeros]
        wv_p = np.zeros((D, 128), f32)
        for hh in range(2):
            hcol = c0 + hh * HD
            wv_p[:, 64 * hh + 0:64 * hh + 32] = Wv[:, hcol:hcol + 32]
            wv_p[:, 64 * hh + 33:64 * hh + 55] = Wv[:, hcol + 32:hcol + HD]

        # rows match AN layout: [d0:32 | dead | d32:54 | dead] per head
        wo_p = np.zeros((128, 216), f32)
        for hh in range(2):
            hrow = c0 + hh * HD
            wo_p[64 * hh + 0:64 * hh + 32, :] = Wo[hrow:hrow + 32, :]
            wo_p[64 * hh + 33:64 * hh + 55, :] = Wo[hrow + 32:hrow + HD, :]

        mkT_p = np.zeros((128, 128), f32)
        mkT_p[0:HD, 0:MEM] = mem_k[b][:, c0:c0 + HD].T
        mkT_p[64:64 + HD, 0:MEM] = mem_k[b][:, c0 + HD:c0 + 2 * HD].T

        mv_p = np.zeros((MEM, 128), f32)
        for hh in range(2):
            hcol = c0 + hh * HD
            mv_p[:, 64 * hh + 0:64 * hh + 32] = mem_v[b][:, hcol:hcol + 32]
            mv_p[:, 64 * hh + 32] = 1.0
            mv_p[:, 64 * hh + 33:64 * hh + 55] = mem_v[b][:, hcol + 32:hcol + HD]

        in_maps.append({
            "xT": np.ascontiguousarray(x[b].T).astype(bf16),
            "wq": wq_p.astype(bf16), "wk": wk_p.astype(bf16),
            "wqr": wqr_p.astype(bf16), "wkr": wkr_p.astype(bf16),
            "wv": wv_p.astype(bf16), "wo": wo_p.astype(bf16),
            "cosT": cosT, "sinT": sinT,
            "memkT": mkT_p.astype(bf16), "memv": mv_p.astype(bf16),
        })
    return in_maps


def get_program():
    global _PROGRAM
    if _PROGRAM is None:
        _PROGRAM = _build_program()
    return _PROGRAM


def kernel(x, mem_k, mem_v, attention_mask, Wqkv, Wo):
    from concourse.bass_utils import run_bass_kernel_spmd

    nc = get_program()
    in_maps = _host_inputs(x, mem_k, mem_v, Wqkv, Wo)
    trace = bool(int(os.environ.get("KB_TRACE", "0")))
    res = run_bass_kernel_spmd(nc, in_maps, core_ids=list(range(NCORES)),
                               trace=trace)
    if trace and res.exec_time_ns is not None:
        print(f"HW exec time: {res.exec_time_ns} ns")
    parts = [res.results[c]["outp"] for c in range(NCORES)]
    out = np.stack([parts[2 * b] + parts[2 * b + 1] for b in range(B)])
    return out.astype(np.float32)
